# revision 17
# baseline (speedup 1.0000x reference)
"""Dead-zone squared-error mean over N=33554432 elements, data-parallel on 8 NeuronCores.

reference:  diff = inputs - targets
            dz   = where(|diff| < 0.1, 0, diff)
            out  = mean(dz * dz)            (scalar float32)

Strategy: shard N across 8 cores (4,194,304 elements each).  The host packs
inputs and targets into one interleaved tensor per core ([tile, P, 2, CHUNK])
so every tile is a single contiguous 2 MiB DMA carrying both operands — one
HBM stream per core, one DMA semaphore per tile.  Per tile:
    d = x - t                 (DVE tensor_sub)
    s = d^2                   (ACT Square)
    r = (s >= 0.01) * s       (DVE scalar_tensor_tensor, fused mask+mul,
                               accum_out -> per-partition partial sum)
The final tile-slot is processed as NSPLIT small sub-tiles so the post-DMA
serial chain is short, and the masked-accumulate of tile i is ordered after
the subtract of tile i+1 (no-sync dep) so the in-order Vector engine never
stalls on the cross-engine square.  Each core returns a [128, NCOL] stats
block; the host sums the partials in float64 and divides by N.
"""

import numpy as np

import concourse.bacc as bacc
import concourse.mybir as mybir
import concourse.tile as tile
from concourse.alu_op_type import AluOpType
from concourse.bass_utils import run_bass_kernel_spmd
from concourse.tile import add_dep_helper

N = 33554432
NCORES = 8
PER_CORE = N // NCORES          # 4194304
P = 128
CHUNK = 2048                    # free elems per bulk tile per operand
NT = PER_CORE // (P * CHUNK)    # 16 tile-slot equivalents per core
NB = NT - 2                     # bulk tiles
NSPLIT = 4                      # sub-tiles per split slot
TAILC = CHUNK // NSPLIT         # 512
NSMALL = 2 * NSPLIT             # small tiles: NSPLIT at head + NSPLIT at tail
NCOL = NB + NSMALL              # stats columns
THRESH_SQ = 0.01                # (dead-zone 0.1)^2

F32 = mybir.dt.float32

_CACHE = {}


def _build_nc():
    nc = bacc.Bacc()
    # interleaved [x | t] per partition row: one contiguous DMA per tile
    xtb = nc.dram_tensor("xtb", [NB, P, 2, CHUNK], F32, kind="ExternalInput")
    xts = nc.dram_tensor("xts", [NSMALL, P, 2, TAILC], F32, kind="ExternalInput")
    out = nc.dram_tensor("out", [P, NCOL], F32, kind="ExternalOutput")

    with tile.TileContext(nc) as tc:
        with (
            tc.tile_pool(name="io", bufs=3) as io_pool,
            tc.tile_pool(name="tmp", bufs=3) as tmp_pool,
            tc.tile_pool(name="stats", bufs=1) as stats_pool,
        ):
            stats = stats_pool.tile([P, NCOL], F32)

            # small and bulk tiles share tags (slots sized to the bulk tile)
            # to keep the allocated-semaphore count low: the per-NEFF
            # sem-clear preamble and the exit sem-reset ladder scale with it.
            def load_and_square(src_ap, c):
                buf = io_pool.tile([P, 2 * CHUNK], F32, tag="io")
                nc.sync.dma_start(out=buf[:, 0 : 2 * c], in_=src_ap)
                d = tmp_pool.tile([P, CHUNK], F32, tag="d")
                tt_ins = nc.vector.tensor_sub(
                    d[:, 0:c], buf[:, 0:c], buf[:, c : 2 * c]
                ).ins
                s = tmp_pool.tile([P, CHUNK], F32, tag="s")
                nc.scalar.activation(
                    s[:, 0:c], d[:, 0:c], mybir.ActivationFunctionType.Square
                )
                return s, tt_ins

            def masked_accum(s, c, col):
                # s = (s >= 0.01) * s in place;
                # stats[:, col] = per-partition sum
                return nc.vector.scalar_tensor_tensor(
                    out=s[:, 0:c],
                    in0=s[:, 0:c],
                    scalar=THRESH_SQ,
                    in1=s[:, 0:c],
                    op0=AluOpType.is_ge,
                    op1=AluOpType.mult,
                    accum_out=stats[:, col : col + 1],
                ).ins

            # NSPLIT small tiles first (Vector starts ~5us earlier), bulk,
            # then NSPLIT small tiles last (short post-DMA serial chain).
            work = [(xts[j], TAILC) for j in range(NSPLIT)]
            work += [(xtb[i], CHUNK) for i in range(NB)]
            work += [(xts[NSPLIT + j], TAILC) for j in range(NSPLIT)]
            pending = None  # (s_tile, c, col)
            for col, (src_ap, c) in enumerate(work):
                s, tt_ins = load_and_square(src_ap, c)
                if pending is not None:
                    stt_ins = masked_accum(*pending)
                    add_dep_helper(
                        stt_ins, tt_ins, sync=False, reason="pipeline skew"
                    )
                pending = (s, c, col)
            masked_accum(*pending)
            nc.sync.dma_start(out=out[:], in_=stats[:])
    nc.finalize()
    return nc


def _pack(inputs: np.ndarray, targets: np.ndarray):
    """Interleave x and t per partition row: per core, bulk [NB, P, 2, CHUNK]
    and small [NSMALL, P, 2, TAILC]."""
    x = np.ascontiguousarray(inputs, dtype=np.float32).reshape(NCORES, PER_CORE)
    t = np.ascontiguousarray(targets, dtype=np.float32).reshape(NCORES, PER_CORE)
    nb_elems = NB * P * CHUNK

    xb = x[:, :nb_elems].reshape(NCORES, NB, P, 1, CHUNK)
    tb = t[:, :nb_elems].reshape(NCORES, NB, P, 1, CHUNK)
    bulk = np.concatenate([xb, tb], axis=3)  # [NCORES, NB, P, 2, CHUNK]

    xs = x[:, nb_elems:].reshape(NCORES, NSMALL, P, 1, TAILC)
    ts = t[:, nb_elems:].reshape(NCORES, NSMALL, P, 1, TAILC)
    small = np.concatenate([xs, ts], axis=3)  # [NCORES, NSMALL, P, 2, TAILC]
    return np.ascontiguousarray(bulk), np.ascontiguousarray(small)


def kernel(inputs: np.ndarray, targets: np.ndarray) -> np.ndarray:
    bulk, tail = _pack(inputs, targets)

    if "nc" not in _CACHE:
        _CACHE["nc"] = _build_nc()
    nc = _CACHE["nc"]

    in_maps = [{"xtb": bulk[c], "xts": tail[c]} for c in range(NCORES)]
    res = run_bass_kernel_spmd(nc, in_maps, list(range(NCORES)))

    total = 0.0
    for r in res.results:
        total += r["out"].astype(np.float64).sum()
    return np.array(total / N, dtype=np.float32)


# revision 26
# speedup vs baseline: 1.0245x; 1.0245x over previous
"""Dead-zone squared-error mean over N=33554432 elements, data-parallel on 8 NeuronCores.

reference:  diff = inputs - targets
            dz   = where(|diff| < 0.1, 0, diff)
            out  = mean(dz * dz)            (scalar float32)

Strategy: shard N across 8 cores (4,194,304 elements each).  The host packs
inputs and targets into one interleaved tensor per core ([tile, P, 2, CHUNK])
so every tile is a single contiguous 2 MiB DMA carrying both operands — one
sequential HBM stream per core, one DMA semaphore per tile.  Per tile:
    d = x - t                 (DVE tensor_sub)
    s = d^2                   (ACT Square)
    r = (s >= 0.01) * s       (DVE scalar_tensor_tensor, fused mask+mul,
                               accum_out -> per-partition partial sum)
The first NSPLIT tiles are small (512 instead of 2048 per operand) so the
Vector engine starts ~5us earlier, and the masked-accumulate of tile i is
ordered after the subtract of tile i+1 so the in-order Vector engine never
stalls on the cross-engine square.  Each core returns a [128, NCOL] stats
block; the host sums the partials in float64 and divides by N.

Two builders produce the identical dataflow:
  _build_nc_raw (default) — hand-scheduled bass with 8 explicit semaphores;
  _build_nc               — TileContext version (~2us slower exit machinery),
selectable with RAW=0 for debugging.

Measured on trn2 (8 cores): ~100us HW exec in a quiet HBM window (the 2 MiB
transfers stream at ~409 GB/s/core = the 820 GB/s per-core-pair domain limit),
~117-121us when the paired cores' streams interfere.  Memory roofline for
2 x 16 MiB/core at the documented 358 GB/s is ~94us.
"""

import numpy as np

import concourse.bacc as bacc
import concourse.mybir as mybir
import concourse.tile as tile
from concourse.alu_op_type import AluOpType
from concourse.bass_utils import run_bass_kernel_spmd
from concourse.tile import add_dep_helper

N = 33554432
NCORES = 8
PER_CORE = N // NCORES          # 4194304
P = 128
CHUNK = 2048                    # free elems per bulk tile per operand
NT = PER_CORE // (P * CHUNK)    # 16 tile-slot equivalents per core
NB = NT - 1                     # bulk tiles
NSPLIT = 4                      # sub-tiles per split slot
TAILC = CHUNK // NSPLIT         # 512
NSMALL = NSPLIT                 # small tiles at the head only
NCOL = NB + NSMALL              # stats columns
THRESH_SQ = 0.01                # (dead-zone 0.1)^2

F32 = mybir.dt.float32

_CACHE = {}


def _build_nc():
    nc = bacc.Bacc()
    # interleaved [x | t] per partition row: one contiguous DMA per tile
    xtb = nc.dram_tensor("xtb", [NB, P, 2, CHUNK], F32, kind="ExternalInput")
    xts = nc.dram_tensor("xts", [NSMALL, P, 2, TAILC], F32, kind="ExternalInput")
    out = nc.dram_tensor("out", [P, NCOL], F32, kind="ExternalOutput")

    with tile.TileContext(nc) as tc:
        with (
            tc.tile_pool(name="io", bufs=3) as io_pool,
            tc.tile_pool(name="tmp", bufs=3) as tmp_pool,
            tc.tile_pool(name="stats", bufs=1) as stats_pool,
        ):
            stats = stats_pool.tile([P, NCOL], F32)

            # small and bulk tiles share tags (slots sized to the bulk tile)
            # to keep the allocated-semaphore count low: the per-NEFF
            # sem-clear preamble and the exit sem-reset ladder scale with it.
            def load_and_square(src_ap, c):
                buf = io_pool.tile([P, 2 * CHUNK], F32, tag="io")
                nc.sync.dma_start(out=buf[:, 0 : 2 * c], in_=src_ap)
                d = tmp_pool.tile([P, CHUNK], F32, tag="d")
                tt_ins = nc.vector.tensor_sub(
                    d[:, 0:c], buf[:, 0:c], buf[:, c : 2 * c]
                ).ins
                s = tmp_pool.tile([P, CHUNK], F32, tag="s")
                nc.scalar.activation(
                    s[:, 0:c], d[:, 0:c], mybir.ActivationFunctionType.Square
                )
                return s, tt_ins

            def masked_accum(s, c, col):
                # s = (s >= 0.01) * s in place;
                # stats[:, col] = per-partition sum
                return nc.vector.scalar_tensor_tensor(
                    out=s[:, 0:c],
                    in0=s[:, 0:c],
                    scalar=THRESH_SQ,
                    in1=s[:, 0:c],
                    op0=AluOpType.is_ge,
                    op1=AluOpType.mult,
                    accum_out=stats[:, col : col + 1],
                ).ins

            # NSPLIT small tiles first (Vector starts ~5us earlier), bulk,
            # then NSPLIT small tiles last (short post-DMA serial chain).
            work = [(xts[j], TAILC) for j in range(NSPLIT)]
            work += [(xtb[i], CHUNK) for i in range(NB)]
            pending = None  # (s_tile, c, col)
            for col, (src_ap, c) in enumerate(work):
                s, tt_ins = load_and_square(src_ap, c)
                if pending is not None:
                    stt_ins = masked_accum(*pending)
                    add_dep_helper(
                        stt_ins, tt_ins, sync=False, reason="pipeline skew"
                    )
                pending = (s, c, col)
            masked_accum(*pending)
            nc.sync.dma_start(out=out[:], in_=stats[:])
    nc.finalize()
    return nc


def _build_nc_raw():
    """Hand-scheduled variant: same dataflow as the Tile version but with four
    explicit semaphores, so the per-NEFF sem-clear preamble and the Tile exit
    machinery (sem-reset ladder + EVSEM butterfly) mostly disappear.

    Slot safety, with B=4 io slots, 2 d slots, 2 s slots:
      - DMA(i) overwrites io[i%B]   -> Sync waits tt_sem >= i-B+1
      - TT(i) overwrites d[i%2]     -> implied: Vector previously waited
                                       act_sem >= i-1 (before STT(i-2))
      - ACT(i) overwrites s[i%2]    -> Scalar waits stt_sem >= i-1
      - STT(i) is in place on s[i%2]
    """
    import contextlib

    B = 4
    nc = bacc.Bacc()
    xtb = nc.dram_tensor("xtb", [NB, P, 2, CHUNK], F32, kind="ExternalInput")
    xts = nc.dram_tensor("xts", [NSMALL, P, 2, TAILC], F32, kind="ExternalInput")
    out = nc.dram_tensor("out", [P, NCOL], F32, kind="ExternalOutput")

    work = [(xts[j], TAILC) for j in range(NSPLIT)]
    work += [(xtb[i], CHUNK) for i in range(NB)]
    ntiles = len(work)

    with contextlib.ExitStack() as ctx:
        io = [
            ctx.enter_context(nc.sbuf_tensor(f"io{k}", [P, 2 * CHUNK], F32))
            for k in range(B)
        ]
        d = [ctx.enter_context(nc.sbuf_tensor(f"d{k}", [P, CHUNK], F32)) for k in range(2)]
        s = [ctx.enter_context(nc.sbuf_tensor(f"s{k}", [P, CHUNK], F32)) for k in range(2)]
        stats = ctx.enter_context(nc.sbuf_tensor("stats", [P, NCOL], F32))
        # One DMA-completion semaphore per io slot: a HWDGE transfer fans out
        # over several queues, so cumulative counting on a single semaphore
        # would let TT(i) pass on partial credits from DMA(i+1).  Transfers
        # sharing a slot sem are serialized by the slot-release chain.
        dma_sems = [
            ctx.enter_context(nc.semaphore(f"dma_sem{k}")) for k in range(B)
        ]
        out_sem = ctx.enter_context(nc.semaphore("out_sem"))
        tt_sem = ctx.enter_context(nc.semaphore("tt_sem"))
        act_sem = ctx.enter_context(nc.semaphore("act_sem"))
        stt_sem = ctx.enter_context(nc.semaphore("stt_sem"))
        block = ctx.enter_context(nc.Block())

        @block.sync
        def _(sync):
            for i, (src_ap, c) in enumerate(work):
                if i >= B:
                    sync.wait_ge(tt_sem, i - B + 1)
                sync.dma_start(out=io[i % B][:, 0 : 2 * c], in_=src_ap).then_inc(
                    dma_sems[i % B], 16
                )
            sync.wait_ge(stt_sem, ntiles)
            sync.dma_start(out=out[:], in_=stats[:]).then_inc(out_sem, 16)
            sync.wait_ge(out_sem, 16)

        @block.vector
        def _(vector):
            def tt(i, c):
                vector.wait_ge(dma_sems[i % B], 16 * (i // B + 1))
                nc.vector.tensor_sub(
                    d[i % 2][:, 0:c], io[i % B][:, 0:c], io[i % B][:, c : 2 * c]
                ).then_inc(tt_sem, 1)

            def stt(i, c):
                vector.wait_ge(act_sem, i + 1)
                nc.vector.scalar_tensor_tensor(
                    out=s[i % 2][:, 0:c],
                    in0=s[i % 2][:, 0:c],
                    scalar=THRESH_SQ,
                    in1=s[i % 2][:, 0:c],
                    op0=AluOpType.is_ge,
                    op1=AluOpType.mult,
                    accum_out=stats[:, i : i + 1],
                ).then_inc(stt_sem, 1)

            tt(0, work[0][1])
            for i in range(1, ntiles):
                tt(i, work[i][1])
                stt(i - 1, work[i - 1][1])
            stt(ntiles - 1, work[ntiles - 1][1])

        @block.scalar
        def _(scalar):
            for i, (_, c) in enumerate(work):
                scalar.wait_ge(tt_sem, i + 1)
                if i >= 2:
                    scalar.wait_ge(stt_sem, i - 1)
                nc.scalar.activation(
                    s[i % 2][:, 0:c],
                    d[i % 2][:, 0:c],
                    mybir.ActivationFunctionType.Square,
                ).then_inc(act_sem, 1)

    nc.finalize()
    return nc


def _pack(inputs: np.ndarray, targets: np.ndarray):
    """Interleave x and t per partition row: per core, bulk [NB, P, 2, CHUNK]
    and small [NSMALL, P, 2, TAILC]."""
    x = np.ascontiguousarray(inputs, dtype=np.float32).reshape(NCORES, PER_CORE)
    t = np.ascontiguousarray(targets, dtype=np.float32).reshape(NCORES, PER_CORE)
    nb_elems = NB * P * CHUNK

    xb = x[:, :nb_elems].reshape(NCORES, NB, P, 1, CHUNK)
    tb = t[:, :nb_elems].reshape(NCORES, NB, P, 1, CHUNK)
    bulk = np.concatenate([xb, tb], axis=3)  # [NCORES, NB, P, 2, CHUNK]

    xs = x[:, nb_elems:].reshape(NCORES, NSMALL, P, 1, TAILC)
    ts = t[:, nb_elems:].reshape(NCORES, NSMALL, P, 1, TAILC)
    small = np.concatenate([xs, ts], axis=3)  # [NCORES, NSMALL, P, 2, TAILC]
    return np.ascontiguousarray(bulk), np.ascontiguousarray(small)


def kernel(inputs: np.ndarray, targets: np.ndarray) -> np.ndarray:
    bulk, tail = _pack(inputs, targets)

    import os

    builder = _build_nc_raw if os.environ.get("RAW", "1") == "1" else _build_nc
    if "nc" not in _CACHE:
        _CACHE["nc"] = builder()
    nc = _CACHE["nc"]

    in_maps = [{"xtb": bulk[c], "xts": tail[c]} for c in range(NCORES)]
    res = run_bass_kernel_spmd(nc, in_maps, list(range(NCORES)))

    total = 0.0
    for r in res.results:
        total += r["out"].astype(np.float64).sum()
    return np.array(total / N, dtype=np.float32)
